# Initial kernel scaffold
#
"""CRF negative log-likelihood loss on 8 TRN2 NeuronCores.

B=512, T=1024, K=64.  Data-parallel over batch: each core gets 64 sequences.

Math (per core, mask == all-ones):
  E = exp(transitions)                      (K,K), resident PE weights
  X~_t = exp(emissions_t - c)               rescale const c folded into exp bias
  fwd:  A~_t = (E^T A~_{t-1}) . X~_t        A~_0 = X~_0 . exp(start)
  bwd:  W~_t = (E  W~_{t+1}) . X~_t         W~_1023 = X~_1023 . exp(end)
  Both have the same "matmul then elementwise-mul" shape, so they run stacked
  in one [128,x] matmul with block-diag weights [[E,0],[0,E^T]] (fwd rows/cols
  0-63, bwd 64-127), meeting in the middle after 512 steps:
    Z~_b = sum_j (E^T A~_511)[j,b] * W~_512[j,b];   logZ_b = ln Z~_b + 1024*c
  numerator_b = start[tag0] + sum_t e[t,tag_t] + sum_t Tr[tag_{t-1},tag_t]
              + end[tag_last]   (indirect-DMA gathers from HBM, exact f32)
  out = sum_b (logZ_b - numerator_b); host sums cores and divides by 512.

Emissions transposed to [k, b] layout (needed since PE contracts over
partitions) via: f32 load -> ACT exp+bf16 cast -> DRAM scratch reorder
(bwd half time-reversed) -> xbar transpose-DMA back to SBUF.
"""

import sys

import numpy as np

for _p in ("/opt/trn_rl_repo",):
    if _p not in sys.path:
        sys.path.insert(0, _p)

import concourse.bass as bass
import concourse.tile as tile
from concourse import bacc, mybir
from concourse.bass import IndirectOffsetOnAxis
from concourse.bass_utils import run_bass_kernel_spmd

F32 = mybir.dt.float32
BF16 = mybir.dt.bfloat16
I32 = mybir.dt.int32
ALU = mybir.AluOpType
ACTF = mybir.ActivationFunctionType

B, T, K = 512, 1024, 64
NCORES = 8
BL = B // NCORES          # 64 sequences per core
HALF = T // 2             # 512 supersteps
NUB = 8                   # number of u-blocks in the transpose pipeline
UW = HALF // NUB          # 64 supersteps per u-block
NCHAIN = 2                # independent recursion chains (b-split)
CW = BL // NCHAIN         # 32 columns per chain
C_RESC = float(np.log(64.0) + 0.5)   # per-step rescale, folded into exp bias
FINAL_CONST = float(BL * T * C_RESC)


def _flip_free(ap, dim):
    """Reverse one free dim of an AP (negative step, offset moved to end)."""
    new = ap.ap.copy()
    step, count = new[dim]
    new_offset = ap.offset + step * (count - 1)
    new[dim] = [-step, count]
    return bass.AP(ap.tensor, new_offset, new)


def _build_kernel(nc, tc):
    em = nc.dram_tensor("emissions", [BL, T, K], F32, kind="ExternalInput")
    tg = nc.dram_tensor("tags_pairs", [BL, 2 * T], I32, kind="ExternalInput")
    tr = nc.dram_tensor("transitions", [K, K], F32, kind="ExternalInput")
    st = nc.dram_tensor("start_transitions", [K], F32, kind="ExternalInput")
    en = nc.dram_tensor("end_transitions", [K], F32, kind="ExternalInput")
    out = nc.dram_tensor("out", [1, 1], F32, kind="ExternalOutput")

    from contextlib import ExitStack

    with ExitStack() as ctx:
        const = ctx.enter_context(tc.tile_pool(name="const", bufs=1))
        scrp = ctx.enter_context(tc.tile_pool(name="scr", bufs=NUB, space="DRAM"))
        enat_p = ctx.enter_context(tc.tile_pool(name="enat", bufs=2))
        xnat_p = ctx.enter_context(tc.tile_pool(name="xnat", bufs=2))
        xt_p = ctx.enter_context(tc.tile_pool(name="xt", bufs=3))
        stA_p = ctx.enter_context(tc.tile_pool(name="stA", bufs=3))
        stB_p = ctx.enter_context(tc.tile_pool(name="stB", bufs=3))
        ps_p = ctx.enter_context(tc.tile_pool(name="ps", bufs=2, space="PSUM"))
        ps2_p = ctx.enter_context(tc.tile_pool(name="ps2", bufs=1, space="PSUM"))
        gat_p = ctx.enter_context(tc.tile_pool(name="gat", bufs=1))
        # ---------------- constants / weights ----------------
        trF = const.tile([K, K], F32)
        nc.sync.dma_start(out=trF[:], in_=tr.ap())
        trE = const.tile([K, K], BF16)
        nc.scalar.activation(trE[:], trF[:], ACTF.Exp)

        # identity for PE transpose
        iotF = const.tile([K, K], I32)
        nc.gpsimd.iota(iotF[:], pattern=[[1, K]], base=0, channel_multiplier=0)
        iotP = const.tile([K, K], I32)
        nc.gpsimd.iota(iotP[:], pattern=[[0, K]], base=0, channel_multiplier=1)
        ident = const.tile([K, K], BF16)
        nc.vector.tensor_tensor(ident[:], iotF[:], iotP[:], ALU.is_equal)

        W = const.tile([128, 128], BF16)
        nc.gpsimd.memset(W[:], 0.0)
        nc.vector.tensor_copy(W[0:K, 0:K], trE[:])
        psT = ps2_p.tile([K, K], BF16)
        nc.tensor.transpose(psT[:], trE[:], ident[:])
        nc.vector.tensor_copy(W[K:128, K:128], psT[:])

        # exp(start) on partitions 0-63, exp(end) on 64-127
        seF = const.tile([128, 1], F32)
        nc.sync.dma_start(out=seF[0:K, :], in_=st.ap().rearrange("(k one) -> k one", one=1))
        nc.sync.dma_start(out=seF[K:128, :], in_=en.ap().rearrange("(k one) -> k one", one=1))
        seE = const.tile([128, 1], F32)
        nc.scalar.activation(seE[:], seF[:], ACTF.Exp)

        ones64 = const.tile([K, 1], F32)
        nc.gpsimd.memset(ones64[:], 1.0)

        biasC = const.tile([BL, 1], F32)
        nc.gpsimd.memset(biasC[:], -C_RESC)

        # ---------------- numerator (indirect gathers) ----------------
        tags32 = gat_p.tile([BL, T], I32)
        tgv = tg.ap().rearrange("b (t two) -> b t two", two=2)
        # two halves: a full-range strided view would AP-merge (b,t) into a
        # 65536-count dim that overflows the 16-bit ISA num_elem field.
        nc.sync.dma_start(
            out=tags32[:, 0:HALF], in_=tgv[:, 0:HALF, 0:1]
        )
        nc.sync.dma_start(
            out=tags32[:, HALF:T], in_=tgv[:, HALF:T, 0:1]
        )
        # gather via a (t, b, k)-permuted view of emissions: permuted dims
        # cannot be merged by AP opt, keeping every dim <= 65535 (ISA field).
        # flat position of (b, t, k) in that view = t*(BL*K) + b*K + k.
        iog = gat_p.tile([BL, T], I32)
        nc.gpsimd.iota(
            iog[:], pattern=[[BL * K, T]], base=0, channel_multiplier=K
        )
        off_em = gat_p.tile([BL, T], I32)
        nc.vector.tensor_tensor(off_em[:], iog[:], tags32[:], ALU.add)
        g_em = gat_p.tile([BL, T], F32)
        nc.gpsimd.indirect_dma_start(
            out=g_em[:],
            out_offset=None,
            in_=em.ap().rearrange("b t k -> t b k"),
            in_offset=IndirectOffsetOnAxis(ap=off_em[:], axis=2),
        )

        t64 = gat_p.tile([BL, T - 1], I32)
        nc.vector.tensor_scalar_mul(t64[:], tags32[:, 0 : T - 1], K)
        off_tr = gat_p.tile([BL, T - 1], I32)
        nc.vector.tensor_tensor(off_tr[:], t64[:], tags32[:, 1:T], ALU.add)
        g_tr = gat_p.tile([BL, T - 1], F32)
        nc.gpsimd.indirect_dma_start(
            out=g_tr[:],
            out_offset=None,
            in_=tr.ap(),
            in_offset=IndirectOffsetOnAxis(ap=off_tr[:], axis=1),
        )

        g_st = gat_p.tile([BL, 2], F32)
        off_se = gat_p.tile([BL, 2], I32)
        nc.vector.tensor_copy(off_se[:, 0:1], tags32[:, 0:1])
        nc.vector.tensor_copy(off_se[:, 1:2], tags32[:, T - 1 : T])
        nc.gpsimd.indirect_dma_start(
            out=g_st[:, 0:1],
            out_offset=None,
            in_=st.ap().rearrange("(k one) -> k one", one=1),
            in_offset=IndirectOffsetOnAxis(ap=off_se[:, 0:1], axis=1),
        )
        nc.gpsimd.indirect_dma_start(
            out=g_st[:, 1:2],
            out_offset=None,
            in_=en.ap().rearrange("(k one) -> k one", one=1),
            in_offset=IndirectOffsetOnAxis(ap=off_se[:, 1:2], axis=1),
        )

        ns1 = gat_p.tile([BL, 1], F32)
        nc.vector.tensor_reduce(ns1[:], g_em[:], mybir.AxisListType.X, ALU.add)
        ns2 = gat_p.tile([BL, 1], F32)
        nc.vector.tensor_reduce(ns2[:], g_tr[:], mybir.AxisListType.X, ALU.add)
        ns3 = gat_p.tile([BL, 1], F32)
        nc.vector.tensor_reduce(ns3[:], g_st[:], mybir.AxisListType.X, ALU.add)
        num = gat_p.tile([BL, 1], F32)
        nc.vector.tensor_tensor(num[:], ns1[:], ns2[:], ALU.add)
        nc.vector.tensor_tensor(num[:], num[:], ns3[:], ALU.add)

        # ---------------- emissions -> X~T pipeline ----------------
        # scratch block ub: [b, ui, h, k] bf16; h=0 holds t = ub*UW+ui,
        # h=1 holds t = 1023 - (ub*UW+ui)  (time-reversed second half).
        blocks = [
            scrp.tile([BL, UW, 2, K], BF16, name=f"blk{i}") for i in range(NUB)
        ]

        load_order = []
        for i in range(NUB // 2):
            load_order += [i, NUB - 1 - i]

        for c in load_order:
            enat = enat_p.tile([BL, 2 * UW * K], F32, tag="enat")
            # h0: t in [c*UW, (c+1)*UW)
            nc.sync.dma_start(
                out=enat[:, 0 : UW * K].rearrange("b (u k) -> b u k", k=K),
                in_=em.ap()[:, c * UW : (c + 1) * UW, :],
            )
            # h1: t in [HALF + c*UW, HALF + (c+1)*UW)
            nc.sync.dma_start(
                out=enat[:, UW * K : 2 * UW * K].rearrange("b (u k) -> b u k", k=K),
                in_=em.ap()[:, HALF + c * UW : HALF + (c + 1) * UW, :],
            )
            xnat = xnat_p.tile([BL, 2 * UW * K], BF16, tag="xnat")
            nc.scalar.activation(xnat[:], enat[:], ACTF.Exp, bias=biasC[:])

            # h0 -> block c, ascending ui
            nc.sync.dma_start(
                out=blocks[c][:, :, 0, :],
                in_=xnat[:, 0 : UW * K].rearrange("b (u k) -> b u k", k=K),
            )
            # h1 -> block NUB-1-c, reversed ui:
            #   src t = HALF + c*UW + tl  ->  u_global = 1023 - t
            #   = (NUB-1-c)*UW + (UW-1-tl)
            src_h1 = xnat[:, UW * K : 2 * UW * K].rearrange(
                "b (u k) -> b u k", k=K
            )
            nc.sync.dma_start(
                out=blocks[NUB - 1 - c][:, :, 1, :],
                in_=_flip_free(src_h1, 1),
            )

        # ---------------- stacked fwd/bwd recursion ----------------
        def xt_slice(tb, ui, q):
            r = tb[:].rearrange("p (b u) -> p b u", u=UW)
            return r[:, q * CW : (q + 1) * CW, ui : ui + 1].rearrange(
                "p b u -> p (b u)"
            )

        stA = None
        stB = None
        for ub in range(NUB):
            xt = xt_p.tile([128, BL * UW], BF16, tag="xt")
            nc.sync.dma_start(
                out=xt[:],
                in_=blocks[ub][:].rearrange("b u h k -> (b u) (h k)"),
                transpose=True,
            )
            for ui in range(UW):
                sigma = ub * UW + ui
                if sigma == 0:
                    stA = stA_p.tile([128, CW], BF16, tag="sA")
                    stB = stB_p.tile([128, CW], BF16, tag="sB")
                    nc.vector.tensor_scalar(
                        stA[:], xt_slice(xt, 0, 0), seE[:], None, op0=ALU.mult
                    )
                    nc.vector.tensor_scalar(
                        stB[:], xt_slice(xt, 0, 1), seE[:], None, op0=ALU.mult
                    )
                    continue
                psA = ps_p.tile([128, CW], F32, tag="pA")
                nc.tensor.matmul(psA[:], W[:], stA[:], start=True, stop=True)
                stA = stA_p.tile([128, CW], BF16, tag="sA")
                nc.vector.tensor_tensor(
                    stA[:], psA[:], xt_slice(xt, ui, 0), ALU.mult
                )
                psB = ps_p.tile([128, CW], F32, tag="pB")
                nc.tensor.matmul(psB[:], W[:], stB[:], start=True, stop=True)
                stB = stB_p.tile([128, CW], BF16, tag="sB")
                nc.vector.tensor_tensor(
                    stB[:], psB[:], xt_slice(xt, ui, 1), ALU.mult
                )

        # ---------------- epilogue ----------------
        # final matmul: top half = E^T A~_511
        psFA = ps_p.tile([128, CW], F32, tag="pA")
        nc.tensor.matmul(psFA[:], W[:], stA[:], start=True, stop=True)
        psFB = ps_p.tile([128, CW], F32, tag="pB")
        nc.tensor.matmul(psFB[:], W[:], stB[:], start=True, stop=True)

        # bring W~_512 (bottom half of final state) to partitions 0-63
        wc = const.tile([K, BL], BF16)
        nc.sync.dma_start(out=wc[:, 0:CW], in_=stA[K:128, :])
        nc.sync.dma_start(out=wc[:, CW:BL], in_=stB[K:128, :])

        V = const.tile([K, BL], F32)
        nc.vector.tensor_tensor(V[:, 0:CW], psFA[0:K, :], wc[:, 0:CW], ALU.mult)
        nc.vector.tensor_tensor(V[:, CW:BL], psFB[0:K, :], wc[:, CW:BL], ALU.mult)

        zrow = ps2_p.tile([1, BL], F32)
        nc.tensor.matmul(zrow[:], ones64[:], V[:], start=True, stop=True)
        lnz = const.tile([1, BL], F32)
        lnzsum = const.tile([1, 1], F32)
        nc.scalar.activation(lnz[:], zrow[:], ACTF.Ln, accum_out=lnzsum[:])

        nps = ps2_p.tile([1, 1], F32)
        nc.tensor.matmul(nps[:], num[:], ones64[:], start=True, stop=True)

        fin = const.tile([1, 1], F32)
        nc.vector.tensor_tensor(fin[:], lnzsum[:], nps[:], ALU.subtract)
        fin2 = const.tile([1, 1], F32)
        nc.vector.tensor_scalar_add(fin2[:], fin[:], FINAL_CONST)
        nc.sync.dma_start(out=out.ap(), in_=fin2[:])


_CACHE = {}


def build_nc():
    if "nc" not in _CACHE:
        nc = bacc.Bacc(
            "TRN2", target_bir_lowering=False, debug=False, num_devices=NCORES
        )
        with tile.TileContext(nc) as tc:
            _build_kernel(nc, tc)
        nc.compile()
        _CACHE["nc"] = nc
    return _CACHE["nc"]


def make_in_maps(emissions, transitions, start_transitions, end_transitions, tags):
    emissions = np.ascontiguousarray(np.asarray(emissions, dtype=np.float32))
    tags = np.ascontiguousarray(np.asarray(tags, dtype=np.int64))
    transitions = np.ascontiguousarray(np.asarray(transitions, dtype=np.float32))
    start_transitions = np.ascontiguousarray(
        np.asarray(start_transitions, dtype=np.float32)
    )
    end_transitions = np.ascontiguousarray(
        np.asarray(end_transitions, dtype=np.float32)
    )
    tags_pairs = tags.view(np.int32).reshape(B, 2 * T)
    in_maps = []
    for i in range(NCORES):
        sl = slice(i * BL, (i + 1) * BL)
        in_maps.append(
            {
                "emissions": np.ascontiguousarray(emissions[sl]),
                "tags_pairs": np.ascontiguousarray(tags_pairs[sl]),
                "transitions": transitions,
                "start_transitions": start_transitions,
                "end_transitions": end_transitions,
            }
        )
    return in_maps


def kernel(emissions, transitions, start_transitions, end_transitions, tags, mask):
    nc = build_nc()
    in_maps = make_in_maps(
        emissions, transitions, start_transitions, end_transitions, tags
    )
    res = run_bass_kernel_spmd(nc, in_maps, core_ids=list(range(NCORES)))
    total = 0.0
    for i in range(NCORES):
        total += float(res.results[i]["out"][0, 0])
    return np.float32(total / B)



# revision 2
# speedup vs baseline: 1.0004x; 1.0004x over previous
"""CRF negative log-likelihood loss on 8 TRN2 NeuronCores.

B=512, T=1024, K=64.  Data-parallel over batch: each core gets 64 sequences.

Math (per core, mask == all-ones):
  E = exp(transitions)                      (K,K), resident PE weights
  X~_t = exp(emissions_t - c)               rescale const c folded into exp bias
  fwd:  A~_t = (E^T A~_{t-1}) . X~_t        A~_0 = X~_0 . exp(start)
  bwd:  W~_t = (E  W~_{t+1}) . X~_t         W~_1023 = X~_1023 . exp(end)
  Both have the same "matmul then elementwise-mul" shape, so they run stacked
  in one [128,x] matmul with block-diag weights [[E,0],[0,E^T]] (fwd rows/cols
  0-63, bwd 64-127), meeting in the middle after 512 steps:
    Z~_b = sum_j (E^T A~_511)[j,b] * W~_512[j,b];   logZ_b = ln Z~_b + 1024*c
  numerator_b = start[tag0] + sum_t e[t,tag_t] + sum_t Tr[tag_{t-1},tag_t]
              + end[tag_last]   (indirect-DMA gathers from HBM, exact f32)
  out = sum_b (logZ_b - numerator_b); host sums cores and divides by 512.

Emissions transposed to [k, b] layout (needed since PE contracts over
partitions) via: f32 load -> ACT exp+bf16 cast -> DRAM scratch reorder
(bwd half time-reversed) -> xbar transpose-DMA back to SBUF.
"""

import sys

import numpy as np

for _p in ("/opt/trn_rl_repo",):
    if _p not in sys.path:
        sys.path.insert(0, _p)

import concourse.bass as bass
import concourse.tile as tile
from concourse import bacc, mybir
from concourse.bass import IndirectOffsetOnAxis
from concourse.bass_utils import run_bass_kernel_spmd

F32 = mybir.dt.float32
BF16 = mybir.dt.bfloat16
I32 = mybir.dt.int32
ALU = mybir.AluOpType
ACTF = mybir.ActivationFunctionType

B, T, K = 512, 1024, 64
NCORES = 8
BL = B // NCORES          # 64 sequences per core
HALF = T // 2             # 512 supersteps
NUB = 8                   # number of u-blocks in the transpose pipeline
UW = HALF // NUB          # 64 supersteps per u-block
NCHAIN = 2                # independent recursion chains (b-split)
CW = BL // NCHAIN         # 32 columns per chain
C_RESC = float(np.log(64.0) + 0.5)   # per-step rescale, folded into exp bias
FINAL_CONST = float(BL * T * C_RESC)


def _flip_free(ap, dim):
    """Reverse one free dim of an AP (negative step, offset moved to end)."""
    new = ap.ap.copy()
    step, count = new[dim]
    new_offset = ap.offset + step * (count - 1)
    new[dim] = [-step, count]
    return bass.AP(ap.tensor, new_offset, new)


def _build_kernel(nc, tc):
    em = nc.dram_tensor("emissions", [BL, T, K], F32, kind="ExternalInput")
    tg = nc.dram_tensor("tags_pairs", [BL, 2 * T], I32, kind="ExternalInput")
    tr = nc.dram_tensor("transitions", [K, K], F32, kind="ExternalInput")
    st = nc.dram_tensor("start_transitions", [K], F32, kind="ExternalInput")
    en = nc.dram_tensor("end_transitions", [K], F32, kind="ExternalInput")
    out = nc.dram_tensor("out", [1, 1], F32, kind="ExternalOutput")

    from contextlib import ExitStack

    with ExitStack() as ctx:
        const = ctx.enter_context(tc.tile_pool(name="const", bufs=1))
        scrp = ctx.enter_context(tc.tile_pool(name="scr", bufs=NUB, space="DRAM"))
        enat_p = ctx.enter_context(tc.tile_pool(name="enat", bufs=2))
        xnat_p = ctx.enter_context(tc.tile_pool(name="xnat", bufs=2))
        xt_p = ctx.enter_context(tc.tile_pool(name="xt", bufs=3))
        stA_p = ctx.enter_context(tc.tile_pool(name="stA", bufs=3))
        stB_p = ctx.enter_context(tc.tile_pool(name="stB", bufs=3))
        ps_p = ctx.enter_context(tc.tile_pool(name="ps", bufs=2, space="PSUM"))
        ps2_p = ctx.enter_context(tc.tile_pool(name="ps2", bufs=1, space="PSUM"))
        gat_p = ctx.enter_context(tc.tile_pool(name="gat", bufs=1))
        # ---------------- constants / weights ----------------
        trF = const.tile([K, K], F32)
        nc.sync.dma_start(out=trF[:], in_=tr.ap())
        trE = const.tile([K, K], BF16)
        nc.scalar.activation(trE[:], trF[:], ACTF.Exp)

        # identity for PE transpose
        iotF = const.tile([K, K], I32)
        nc.gpsimd.iota(iotF[:], pattern=[[1, K]], base=0, channel_multiplier=0)
        iotP = const.tile([K, K], I32)
        nc.gpsimd.iota(iotP[:], pattern=[[0, K]], base=0, channel_multiplier=1)
        ident = const.tile([K, K], BF16)
        nc.vector.tensor_tensor(ident[:], iotF[:], iotP[:], ALU.is_equal)

        W = const.tile([128, 128], BF16)
        nc.gpsimd.memset(W[:], 0.0)
        nc.vector.tensor_copy(W[0:K, 0:K], trE[:])
        psT = ps2_p.tile([K, K], BF16)
        nc.tensor.transpose(psT[:], trE[:], ident[:])
        nc.vector.tensor_copy(W[K:128, K:128], psT[:])

        # exp(start) on partitions 0-63, exp(end) on 64-127
        seF = const.tile([128, 1], F32)
        nc.sync.dma_start(out=seF[0:K, :], in_=st.ap().rearrange("(k one) -> k one", one=1))
        nc.sync.dma_start(out=seF[K:128, :], in_=en.ap().rearrange("(k one) -> k one", one=1))
        seE = const.tile([128, 1], F32)
        nc.scalar.activation(seE[:], seF[:], ACTF.Exp)

        ones64 = const.tile([K, 1], F32)
        nc.gpsimd.memset(ones64[:], 1.0)

        biasC = const.tile([BL, 1], F32)
        nc.gpsimd.memset(biasC[:], -C_RESC)

        # ---------------- numerator (indirect gathers) ----------------
        tags32 = gat_p.tile([BL, T], I32)
        tgv = tg.ap().rearrange("b (t two) -> b t two", two=2)
        # two halves: a full-range strided view would AP-merge (b,t) into a
        # 65536-count dim that overflows the 16-bit ISA num_elem field.
        nc.sync.dma_start(
            out=tags32[:, 0:HALF], in_=tgv[:, 0:HALF, 0:1]
        )
        nc.sync.dma_start(
            out=tags32[:, HALF:T], in_=tgv[:, HALF:T, 0:1]
        )
        # gather via a (t, b, k)-permuted view of emissions: permuted dims
        # cannot be merged by AP opt, keeping every dim <= 65535 (ISA field).
        # flat position of (b, t, k) in that view = t*(BL*K) + b*K + k.
        iog = gat_p.tile([BL, T], I32)
        nc.gpsimd.iota(
            iog[:], pattern=[[BL * K, T]], base=0, channel_multiplier=K
        )
        off_em = gat_p.tile([BL, T], I32)
        nc.vector.tensor_tensor(off_em[:], iog[:], tags32[:], ALU.add)
        g_em = gat_p.tile([BL, T], F32)
        nc.gpsimd.indirect_dma_start(
            out=g_em[:],
            out_offset=None,
            in_=em.ap().rearrange("b t k -> t b k"),
            in_offset=IndirectOffsetOnAxis(ap=off_em[:], axis=2),
        )

        t64 = gat_p.tile([BL, T - 1], I32)
        nc.vector.tensor_scalar_mul(t64[:], tags32[:, 0 : T - 1], K)
        off_tr = gat_p.tile([BL, T - 1], I32)
        nc.vector.tensor_tensor(off_tr[:], t64[:], tags32[:, 1:T], ALU.add)
        g_tr = gat_p.tile([BL, T - 1], F32)
        nc.gpsimd.indirect_dma_start(
            out=g_tr[:],
            out_offset=None,
            in_=tr.ap(),
            in_offset=IndirectOffsetOnAxis(ap=off_tr[:], axis=1),
        )

        g_st = gat_p.tile([BL, 2], F32)
        off_se = gat_p.tile([BL, 2], I32)
        nc.vector.tensor_copy(off_se[:, 0:1], tags32[:, 0:1])
        nc.vector.tensor_copy(off_se[:, 1:2], tags32[:, T - 1 : T])
        nc.gpsimd.indirect_dma_start(
            out=g_st[:, 0:1],
            out_offset=None,
            in_=st.ap().rearrange("(k one) -> k one", one=1),
            in_offset=IndirectOffsetOnAxis(ap=off_se[:, 0:1], axis=1),
        )
        nc.gpsimd.indirect_dma_start(
            out=g_st[:, 1:2],
            out_offset=None,
            in_=en.ap().rearrange("(k one) -> k one", one=1),
            in_offset=IndirectOffsetOnAxis(ap=off_se[:, 1:2], axis=1),
        )

        ns1 = gat_p.tile([BL, 1], F32)
        nc.vector.tensor_reduce(ns1[:], g_em[:], mybir.AxisListType.X, ALU.add)
        ns2 = gat_p.tile([BL, 1], F32)
        nc.vector.tensor_reduce(ns2[:], g_tr[:], mybir.AxisListType.X, ALU.add)
        ns3 = gat_p.tile([BL, 1], F32)
        nc.vector.tensor_reduce(ns3[:], g_st[:], mybir.AxisListType.X, ALU.add)
        num = gat_p.tile([BL, 1], F32)
        nc.vector.tensor_tensor(num[:], ns1[:], ns2[:], ALU.add)
        nc.vector.tensor_tensor(num[:], num[:], ns3[:], ALU.add)

        # ---------------- emissions -> X~T pipeline ----------------
        # scratch block ub: [b, ui, h, k] bf16; h=0 holds t = ub*UW+ui,
        # h=1 holds t = 1023 - (ub*UW+ui)  (time-reversed second half).
        blocks = [
            scrp.tile([BL, UW, 2, K], BF16, name=f"blk{i}") for i in range(NUB)
        ]

        load_order = []
        for i in range(NUB // 2):
            load_order += [i, NUB - 1 - i]

        for c in load_order:
            enat = enat_p.tile([BL, 2 * UW * K], F32, tag="enat")
            # h0: t in [c*UW, (c+1)*UW)
            nc.sync.dma_start(
                out=enat[:, 0 : UW * K].rearrange("b (u k) -> b u k", k=K),
                in_=em.ap()[:, c * UW : (c + 1) * UW, :],
            )
            # h1: t in [HALF + c*UW, HALF + (c+1)*UW)
            nc.sync.dma_start(
                out=enat[:, UW * K : 2 * UW * K].rearrange("b (u k) -> b u k", k=K),
                in_=em.ap()[:, HALF + c * UW : HALF + (c + 1) * UW, :],
            )
            xnat = xnat_p.tile([BL, 2 * UW * K], BF16, tag="xnat")
            nc.scalar.activation(xnat[:], enat[:], ACTF.Exp, bias=biasC[:])

            # h0 -> block c, ascending ui
            nc.sync.dma_start(
                out=blocks[c][:, :, 0, :],
                in_=xnat[:, 0 : UW * K].rearrange("b (u k) -> b u k", k=K),
            )
            # h1 -> block NUB-1-c, reversed ui:
            #   src t = HALF + c*UW + tl  ->  u_global = 1023 - t
            #   = (NUB-1-c)*UW + (UW-1-tl)
            src_h1 = xnat[:, UW * K : 2 * UW * K].rearrange(
                "b (u k) -> b u k", k=K
            )
            nc.sync.dma_start(
                out=blocks[NUB - 1 - c][:, :, 1, :],
                in_=_flip_free(src_h1, 1),
            )

        # ---------------- stacked fwd/bwd recursion ----------------
        def xt_slice(tb, ui, q):
            r = tb[:].rearrange("p (b u) -> p b u", u=UW)
            return r[:, q * CW : (q + 1) * CW, ui : ui + 1].rearrange(
                "p b u -> p (b u)"
            )

        stA = None
        stB = None
        for ub in range(NUB):
            xt = xt_p.tile([128, BL * UW], BF16, tag="xt")
            nc.sync.dma_start(
                out=xt[:],
                in_=blocks[ub][:].rearrange("b u h k -> (b u) (h k)"),
                transpose=True,
            )
            for ui in range(UW):
                sigma = ub * UW + ui
                if sigma == 0:
                    stA = stA_p.tile([128, CW], BF16, tag="sA")
                    stB = stB_p.tile([128, CW], BF16, tag="sB")
                    nc.vector.tensor_scalar(
                        stA[:], xt_slice(xt, 0, 0), seE[:], None, op0=ALU.mult
                    )
                    nc.vector.tensor_scalar(
                        stB[:], xt_slice(xt, 0, 1), seE[:], None, op0=ALU.mult
                    )
                    continue
                psA = ps_p.tile([128, CW], F32, tag="pA")
                nc.tensor.matmul(psA[:], W[:], stA[:], start=True, stop=True)
                stA = stA_p.tile([128, CW], BF16, tag="sA")
                nc.vector.tensor_tensor(
                    stA[:], psA[:], xt_slice(xt, ui, 0), ALU.mult
                )
                psB = ps_p.tile([128, CW], F32, tag="pB")
                nc.tensor.matmul(psB[:], W[:], stB[:], start=True, stop=True)
                stB = stB_p.tile([128, CW], BF16, tag="sB")
                nc.vector.tensor_tensor(
                    stB[:], psB[:], xt_slice(xt, ui, 1), ALU.mult
                )

        # ---------------- epilogue ----------------
        # final matmul: top half = E^T A~_511
        psFA = ps_p.tile([128, CW], F32, tag="pA")
        nc.tensor.matmul(psFA[:], W[:], stA[:], start=True, stop=True)
        psFB = ps_p.tile([128, CW], F32, tag="pB")
        nc.tensor.matmul(psFB[:], W[:], stB[:], start=True, stop=True)

        # bring W~_512 (bottom half of final state) to partitions 0-63
        wc = const.tile([K, BL], BF16)
        nc.sync.dma_start(out=wc[:, 0:CW], in_=stA[K:128, :])
        nc.sync.dma_start(out=wc[:, CW:BL], in_=stB[K:128, :])

        V = const.tile([K, BL], F32)
        nc.vector.tensor_tensor(V[:, 0:CW], psFA[0:K, :], wc[:, 0:CW], ALU.mult)
        nc.vector.tensor_tensor(V[:, CW:BL], psFB[0:K, :], wc[:, CW:BL], ALU.mult)

        zrow = ps2_p.tile([1, BL], F32)
        nc.tensor.matmul(zrow[:], ones64[:], V[:], start=True, stop=True)
        lnz = const.tile([1, BL], F32)
        lnzsum = const.tile([1, 1], F32)
        nc.scalar.activation(lnz[:], zrow[:], ACTF.Ln, accum_out=lnzsum[:])

        nps = ps2_p.tile([1, 1], F32)
        nc.tensor.matmul(nps[:], num[:], ones64[:], start=True, stop=True)

        fin = const.tile([1, 1], F32)
        nc.vector.tensor_tensor(fin[:], lnzsum[:], nps[:], ALU.subtract)
        fin2 = const.tile([1, 1], F32)
        nc.vector.tensor_scalar_add(fin2[:], fin[:], FINAL_CONST)
        nc.sync.dma_start(out=out.ap(), in_=fin2[:])


_CACHE = {}


def build_nc():
    if "nc" not in _CACHE:
        nc = bacc.Bacc(
            "TRN2", target_bir_lowering=False, debug=False, num_devices=NCORES
        )
        with tile.TileContext(nc) as tc:
            _build_kernel(nc, tc)
        nc.compile()
        _CACHE["nc"] = nc
    return _CACHE["nc"]


def make_in_maps(emissions, transitions, start_transitions, end_transitions, tags):
    emissions = np.ascontiguousarray(np.asarray(emissions, dtype=np.float32))
    tags = np.ascontiguousarray(np.asarray(tags, dtype=np.int64))
    transitions = np.ascontiguousarray(np.asarray(transitions, dtype=np.float32))
    start_transitions = np.ascontiguousarray(
        np.asarray(start_transitions, dtype=np.float32)
    )
    end_transitions = np.ascontiguousarray(
        np.asarray(end_transitions, dtype=np.float32)
    )
    tags_pairs = tags.view(np.int32).reshape(B, 2 * T)
    in_maps = []
    for i in range(NCORES):
        sl = slice(i * BL, (i + 1) * BL)
        in_maps.append(
            {
                "emissions": np.ascontiguousarray(emissions[sl]),
                "tags_pairs": np.ascontiguousarray(tags_pairs[sl]),
                "transitions": transitions,
                "start_transitions": start_transitions,
                "end_transitions": end_transitions,
            }
        )
    return in_maps


def reduce_results(results):
    total = 0.0
    for i in range(NCORES):
        total += float(results[i]["out"][0, 0])
    return np.float32(total / B)


def kernel(emissions, transitions, start_transitions, end_transitions, tags, mask):
    nc = build_nc()
    in_maps = make_in_maps(
        emissions, transitions, start_transitions, end_transitions, tags
    )
    res = run_bass_kernel_spmd(nc, in_maps, core_ids=list(range(NCORES)))
    return reduce_results(res.results)



# revision 5
# speedup vs baseline: 1.2902x; 1.2896x over previous
"""CRF negative log-likelihood loss on 8 TRN2 NeuronCores.

B=512, T=1024, K=64.  Data-parallel over batch: each core gets 64 sequences.

Math (per core, mask == all-ones):
  E = exp(transitions)                      (K,K), resident PE weights
  X~_t = exp(emissions_t - c)               rescale const c folded into exp bias
  fwd:  A~_t = (E^T A~_{t-1}) . X~_t        A~_0 = X~_0 . exp(start)
  bwd:  W~_t = (E  W~_{t+1}) . X~_t         W~_1023 = X~_1023 . exp(end)
  Both run stacked in one [128,x] matmul with block-diag weights
  [[E,0],[0,E^T]] (fwd rows/cols 0-63, bwd 64-127), meeting in the middle
  after 512 supersteps:
    Z~_b = sum_j (E^T A~_511)[j,b] * W~_512[j,b];   logZ_b = ln Z~_b + 1024*c
  numerator_b = start[tag0] + sum_t e[t,tag_t] + sum_t Tr[tag_{t-1},tag_t]
              + end[tag_last]   (indirect-DMA gathers from HBM, exact f32)
  out = sum_b (logZ_b - numerator_b); host sums cores and divides by 512.

v2 layout notes (all DMAs big+contiguous, DVE operands contiguous):
  - tags loaded as one contiguous [BL, 2T] i32 block; low words extracted
    with strided engine reads (no 4-byte DMA descriptors).
  - scratch block per u-block tb is [2, BL, UW, K] bf16 (h-major): both h
    planes written as fully contiguous 0.5 MiB DMAs; the bwd plane's time
    reversal happens on the SBUF read side of the write (penalty-free).
  - transpose-DMA reads rows (u, b) so xt columns are u-major: the
    per-superstep slice xt[:, u*BL + q*CW ...] is contiguous.
  - constant weights stay resident in the PE array: one explicit ldweights,
    all matmuls issued with ldweights=False.
"""

import sys

import numpy as np

for _p in ("/opt/trn_rl_repo",):
    if _p not in sys.path:
        sys.path.insert(0, _p)

import concourse.bass as bass
import concourse.tile as tile
from concourse import bacc, mybir
from concourse.bass import IndirectOffsetOnAxis
from concourse.bass_utils import run_bass_kernel_spmd

F32 = mybir.dt.float32
BF16 = mybir.dt.bfloat16
I32 = mybir.dt.int32
ALU = mybir.AluOpType
ACTF = mybir.ActivationFunctionType

B, T, K = 512, 1024, 64
NCORES = 8
BL = B // NCORES          # 64 sequences per core
HALF = T // 2             # 512 supersteps
NUB = 8                   # number of u-blocks in the pipeline
UW = HALF // NUB          # 64 supersteps per u-block
NCHAIN = 2                # independent recursion chains (b-split)
CW = BL // NCHAIN         # 32 columns per chain
C_RESC = float(np.log(64.0) + 0.5)   # per-step rescale, folded into exp bias
FINAL_CONST = float(BL * T * C_RESC)

USE_LDW_ONCE = True       # one explicit ldweights; matmuls skip self-load


def _flip_free(ap, dim):
    """Reverse one free dim of an AP (negative step, offset moved to end)."""
    new = ap.ap.copy()
    step, count = new[dim]
    new_offset = ap.offset + step * (count - 1)
    new[dim] = [-step, count]
    return bass.AP(ap.tensor, new_offset, new)


def _mm(nc, out, w, rhs):
    inst = nc.tensor.matmul(out, w, rhs, start=True, stop=True)
    if USE_LDW_ONCE:
        inst.ldweights = False
    return inst


def _build_kernel(nc, tc):
    em = nc.dram_tensor("emissions", [BL, T, K], F32, kind="ExternalInput")
    tg = nc.dram_tensor("tags_pairs", [BL, 2 * T], I32, kind="ExternalInput")
    tr = nc.dram_tensor("transitions", [K, K], F32, kind="ExternalInput")
    st = nc.dram_tensor("start_transitions", [K], F32, kind="ExternalInput")
    en = nc.dram_tensor("end_transitions", [K], F32, kind="ExternalInput")
    out = nc.dram_tensor("out", [1, 1], F32, kind="ExternalOutput")

    from contextlib import ExitStack

    with ExitStack() as ctx:
        const = ctx.enter_context(tc.tile_pool(name="const", bufs=1))
        scrp = ctx.enter_context(tc.tile_pool(name="scr", bufs=NUB, space="DRAM"))
        enat_p = ctx.enter_context(tc.tile_pool(name="enat", bufs=2))
        xnat_p = ctx.enter_context(tc.tile_pool(name="xnat", bufs=2))
        xt_p = ctx.enter_context(tc.tile_pool(name="xt", bufs=3))
        stA_p = ctx.enter_context(tc.tile_pool(name="stA", bufs=3))
        stB_p = ctx.enter_context(tc.tile_pool(name="stB", bufs=3))
        ps_p = ctx.enter_context(tc.tile_pool(name="ps", bufs=2, space="PSUM"))
        ps2_p = ctx.enter_context(tc.tile_pool(name="ps2", bufs=1, space="PSUM"))
        gat_p = ctx.enter_context(tc.tile_pool(name="gat", bufs=1))

        # ---------------- constants / weights ----------------
        trF = const.tile([K, K], F32)
        nc.sync.dma_start(out=trF[:], in_=tr.ap())
        trE = const.tile([K, K], BF16)
        nc.scalar.activation(trE[:], trF[:], ACTF.Exp)

        # identity for PE transpose
        iotF = const.tile([K, K], I32)
        nc.gpsimd.iota(iotF[:], pattern=[[1, K]], base=0, channel_multiplier=0)
        iotP = const.tile([K, K], I32)
        nc.gpsimd.iota(iotP[:], pattern=[[0, K]], base=0, channel_multiplier=1)
        ident = const.tile([K, K], BF16)
        nc.vector.tensor_tensor(ident[:], iotF[:], iotP[:], ALU.is_equal)

        W = const.tile([128, 128], BF16)
        nc.gpsimd.memset(W[:], 0.0)
        nc.vector.tensor_copy(W[0:K, 0:K], trE[:])
        psT = ps2_p.tile([K, K], BF16)
        nc.tensor.transpose(psT[:], trE[:], ident[:])
        nc.vector.tensor_copy(W[K:128, K:128], psT[:])
        if USE_LDW_ONCE:
            nc.tensor.ldweights(W[:])

        # exp(start) on partitions 0-63, exp(end) on 64-127
        seF = const.tile([128, 1], F32)
        nc.sync.dma_start(out=seF[0:K, :], in_=st.ap().rearrange("(k one) -> k one", one=1))
        nc.sync.dma_start(out=seF[K:128, :], in_=en.ap().rearrange("(k one) -> k one", one=1))
        seE = const.tile([128, 1], F32)
        nc.scalar.activation(seE[:], seF[:], ACTF.Exp)

        ones64 = const.tile([K, 1], F32)
        nc.gpsimd.memset(ones64[:], 1.0)

        biasC = const.tile([BL, 1], F32)
        nc.gpsimd.memset(biasC[:], -C_RESC)

        # ---------------- numerator (indirect gathers) ----------------
        # one contiguous load of the int64 tag pairs; low words are read with
        # stride-2 engine APs (no per-element DMA descriptors).
        tgp = gat_p.tile([BL, 2 * T], I32)
        nc.sync.dma_start(out=tgp[:], in_=tg.ap())
        tlow = tgp[:].rearrange("b (t two) -> b t two", two=2)[:, :, 0:1].rearrange(
            "b t one -> b (t one)"
        )

        # gather via a (t, b, k)-permuted view of emissions: permuted dims
        # cannot be merged by AP opt, keeping every dim <= 65535 (ISA field).
        # flat position of (b, t, k) in that view = t*(BL*K) + b*K + k.
        iog = gat_p.tile([BL, T], I32)
        nc.gpsimd.iota(
            iog[:], pattern=[[BL * K, T]], base=0, channel_multiplier=K
        )
        off_em = gat_p.tile([BL, T], I32)
        nc.vector.tensor_tensor(off_em[:], iog[:], tlow, ALU.add)
        g_em = gat_p.tile([BL, T], F32)
        nc.gpsimd.indirect_dma_start(
            out=g_em[:],
            out_offset=None,
            in_=em.ap().rearrange("b t k -> t b k"),
            in_offset=IndirectOffsetOnAxis(ap=off_em[:], axis=2),
        )

        t64 = gat_p.tile([BL, T - 1], I32)
        nc.vector.tensor_scalar_mul(t64[:], tlow[:, 0 : T - 1], K)
        off_tr = gat_p.tile([BL, T - 1], I32)
        nc.vector.tensor_tensor(off_tr[:], t64[:], tlow[:, 1:T], ALU.add)
        g_tr = gat_p.tile([BL, T - 1], F32)
        nc.gpsimd.indirect_dma_start(
            out=g_tr[:],
            out_offset=None,
            in_=tr.ap(),
            in_offset=IndirectOffsetOnAxis(ap=off_tr[:], axis=1),
        )

        g_st = gat_p.tile([BL, 2], F32)
        off_se = gat_p.tile([BL, 2], I32)
        nc.vector.tensor_copy(off_se[:, 0:1], tlow[:, 0:1])
        nc.vector.tensor_copy(off_se[:, 1:2], tlow[:, T - 1 : T])
        nc.gpsimd.indirect_dma_start(
            out=g_st[:, 0:1],
            out_offset=None,
            in_=st.ap().rearrange("(k one) -> k one", one=1),
            in_offset=IndirectOffsetOnAxis(ap=off_se[:, 0:1], axis=1),
        )
        nc.gpsimd.indirect_dma_start(
            out=g_st[:, 1:2],
            out_offset=None,
            in_=en.ap().rearrange("(k one) -> k one", one=1),
            in_offset=IndirectOffsetOnAxis(ap=off_se[:, 1:2], axis=1),
        )

        ns1 = gat_p.tile([BL, 1], F32)
        nc.vector.tensor_reduce(ns1[:], g_em[:], mybir.AxisListType.X, ALU.add)
        ns2 = gat_p.tile([BL, 1], F32)
        nc.vector.tensor_reduce(ns2[:], g_tr[:], mybir.AxisListType.X, ALU.add)
        ns3 = gat_p.tile([BL, 1], F32)
        nc.vector.tensor_reduce(ns3[:], g_st[:], mybir.AxisListType.X, ALU.add)
        num = gat_p.tile([BL, 1], F32)
        nc.vector.tensor_tensor(num[:], ns1[:], ns2[:], ALU.add)
        nc.vector.tensor_tensor(num[:], num[:], ns3[:], ALU.add)

        # ---------------- emissions -> X~T pipeline ----------------
        # scratch block tb: [b, u, h, k] bf16 (baseline layout, 2D-transposable
        # view [(b u), (h k)]).  The h-interleave happens inside the ACT exp
        # (strided engine writes are free), so each block is written to DRAM
        # as ONE fully contiguous 1 MiB DMA.  h=0 holds t = tb*UW+u; h=1 holds
        # t = 1023-(tb*UW+u) (em range [HALF+(NUB-1-tb)*UW, HALF+(NUB-tb)*UW)
        # u-reversed on the ACT input side).
        blocks = [
            scrp.tile([BL, UW, 2, K], BF16, name=f"blk{i}") for i in range(NUB)
        ]

        for tb in range(NUB):
            enat = enat_p.tile([BL, 2 * UW * K], F32, tag="enat")
            nc.sync.dma_start(
                out=enat[:, 0 : UW * K].rearrange("b (u k) -> b u k", k=K),
                in_=em.ap()[:, tb * UW : (tb + 1) * UW, :],
            )
            nc.sync.dma_start(
                out=enat[:, UW * K : 2 * UW * K].rearrange("b (u k) -> b u k", k=K),
                in_=em.ap()[
                    :, HALF + (NUB - 1 - tb) * UW : HALF + (NUB - tb) * UW, :
                ],
            )
            xnat = xnat_p.tile([BL, UW * 2 * K], BF16, tag="xnat")
            xnat_v = xnat[:].rearrange("b (u h k) -> b u h k", h=2, k=K)
            nc.scalar.activation(
                xnat_v[:, :, 0, :],
                enat[:, 0 : UW * K].rearrange("b (u k) -> b u k", k=K),
                ACTF.Exp,
                bias=biasC[:],
            )
            nc.scalar.activation(
                xnat_v[:, :, 1, :],
                _flip_free(
                    enat[:, UW * K : 2 * UW * K].rearrange("b (u k) -> b u k", k=K),
                    1,
                ),
                ACTF.Exp,
                bias=biasC[:],
            )
            # one contiguous 1 MiB write per block
            nc.scalar.dma_start(out=blocks[tb][:], in_=xnat[:])

        # ---------------- stacked fwd/bwd recursion ----------------
        def xt_slice(tb_, ui, q):
            r = tb_[:].rearrange("p (b u) -> p b u", u=UW)
            return r[:, q * CW : (q + 1) * CW, ui : ui + 1].rearrange(
                "p b u -> p (b u)"
            )

        stA = None
        stB = None
        for ub in range(NUB):
            xt = xt_p.tile([128, BL * UW], BF16, tag="xt")
            nc.sync.dma_start(
                out=xt[:],
                in_=blocks[ub][:].rearrange("b u h k -> (b u) (h k)"),
                transpose=True,
            )
            for ui in range(UW):
                sigma = ub * UW + ui
                if sigma == 0:
                    stA = stA_p.tile([128, CW], BF16, tag="sA")
                    stB = stB_p.tile([128, CW], BF16, tag="sB")
                    nc.vector.tensor_scalar(
                        stA[:], xt_slice(xt, 0, 0), seE[:], None, op0=ALU.mult
                    )
                    nc.vector.tensor_scalar(
                        stB[:], xt_slice(xt, 0, 1), seE[:], None, op0=ALU.mult
                    )
                    continue
                psA = ps_p.tile([128, CW], F32, tag="pA")
                _mm(nc, psA[:], W[:], stA[:])
                stA = stA_p.tile([128, CW], BF16, tag="sA")
                nc.vector.tensor_tensor(
                    stA[:], psA[:], xt_slice(xt, ui, 0), ALU.mult
                )
                psB = ps_p.tile([128, CW], F32, tag="pB")
                _mm(nc, psB[:], W[:], stB[:])
                stB = stB_p.tile([128, CW], BF16, tag="sB")
                nc.vector.tensor_tensor(
                    stB[:], psB[:], xt_slice(xt, ui, 1), ALU.mult
                )

        # ---------------- epilogue ----------------
        # final matmul: top half = E^T A~_511
        psFA = ps_p.tile([128, CW], F32, tag="pA")
        _mm(nc, psFA[:], W[:], stA[:])
        psFB = ps_p.tile([128, CW], F32, tag="pB")
        _mm(nc, psFB[:], W[:], stB[:])

        # bring W~_512 (bottom half of final state) to partitions 0-63
        wc = const.tile([K, BL], BF16)
        nc.sync.dma_start(out=wc[:, 0:CW], in_=stA[K:128, :])
        nc.sync.dma_start(out=wc[:, CW:BL], in_=stB[K:128, :])

        V = const.tile([K, BL], F32)
        nc.vector.tensor_tensor(V[:, 0:CW], psFA[0:K, :], wc[:, 0:CW], ALU.mult)
        nc.vector.tensor_tensor(V[:, CW:BL], psFB[0:K, :], wc[:, CW:BL], ALU.mult)

        zrow = ps2_p.tile([1, BL], F32)
        _mm(nc, zrow[:], ones64[:], V[:])
        lnz = const.tile([1, BL], F32)
        lnzsum = const.tile([1, 1], F32)
        nc.scalar.activation(lnz[:], zrow[:], ACTF.Ln, accum_out=lnzsum[:])

        nps = ps2_p.tile([1, 1], F32)
        _mm(nc, nps[:], num[:], ones64[:])

        fin = const.tile([1, 1], F32)
        nc.vector.tensor_tensor(fin[:], lnzsum[:], nps[:], ALU.subtract)
        fin2 = const.tile([1, 1], F32)
        nc.vector.tensor_scalar_add(fin2[:], fin[:], FINAL_CONST)
        nc.sync.dma_start(out=out.ap(), in_=fin2[:])


_CACHE = {}


def build_nc():
    if "nc" not in _CACHE:
        nc = bacc.Bacc(
            "TRN2", target_bir_lowering=False, debug=False, num_devices=NCORES
        )
        with tile.TileContext(nc) as tc:
            _build_kernel(nc, tc)
        nc.compile()
        _CACHE["nc"] = nc
    return _CACHE["nc"]


def make_in_maps(emissions, transitions, start_transitions, end_transitions, tags):
    emissions = np.ascontiguousarray(np.asarray(emissions, dtype=np.float32))
    tags = np.ascontiguousarray(np.asarray(tags, dtype=np.int64))
    transitions = np.ascontiguousarray(np.asarray(transitions, dtype=np.float32))
    start_transitions = np.ascontiguousarray(
        np.asarray(start_transitions, dtype=np.float32)
    )
    end_transitions = np.ascontiguousarray(
        np.asarray(end_transitions, dtype=np.float32)
    )
    tags_pairs = tags.view(np.int32).reshape(B, 2 * T)
    in_maps = []
    for i in range(NCORES):
        sl = slice(i * BL, (i + 1) * BL)
        in_maps.append(
            {
                "emissions": np.ascontiguousarray(emissions[sl]),
                "tags_pairs": np.ascontiguousarray(tags_pairs[sl]),
                "transitions": transitions,
                "start_transitions": start_transitions,
                "end_transitions": end_transitions,
            }
        )
    return in_maps


def reduce_results(results):
    total = 0.0
    for i in range(NCORES):
        total += float(results[i]["out"][0, 0])
    return np.float32(total / B)


def kernel(emissions, transitions, start_transitions, end_transitions, tags, mask):
    nc = build_nc()
    in_maps = make_in_maps(
        emissions, transitions, start_transitions, end_transitions, tags
    )
    res = run_bass_kernel_spmd(nc, in_maps, core_ids=list(range(NCORES)))
    return reduce_results(res.results)
